# revision 3
# baseline (speedup 1.0000x reference)
"""Trainium2 Bass kernel for nn_MixtureOfExperts (B=8192, D=1024, E=12, H=512, O=256).

Strategy:
- Data-parallel over 8 NeuronCores: each core processes 1024 rows of x; all
  weights replicated. Host gathers/concats core outputs.
- Host-side prep: eval-mode BatchNorm (which follows each ReLU) is folded into
  the NEXT layer's weights and bias:  bn(relu(z)) = s*relu(z) + t  with
  s = g/sqrt(v+eps) > 0, t = b - m*s, so
      bn(relu(z)) @ W + c  ==  relu(z) @ (diag(s) W) + (c + t @ W).
  x is pre-transposed and all weights pre-tiled on host into the exact SBUF
  layout ([128 part, chunk, free] with per-partition-contiguous DRAM bytes) so
  every big DMA is a fully contiguous copy.
- All big matmuls bf16 (same PE rate as fp32r but half the DMA traffic and
  2x-faster FWL weight loads; rel err ~5e-3 vs 2e-2 budget). Small gate-tail
  matmuls (g3 / bias-init) stay fp32r.
- Startup: expert-0 L1 runs FIRST in dc-streaming order with 8 open PSUM
  accumulation groups, consuming x[dc] + w1[dc] chunk pairs as they land
  (the two DMA queues carry each pair on opposite queues).  That keeps the
  PE saturated with real work during the DMA-bound window.  Two fp32 filler
  matmuls cover the pre-first-chunk gap so the HAM clock-gate stays released.
  The gate network runs densely right after (x is resident by then).
- Layers 1-3 feature-major; layer 4 batch-major (stationary = h3T slice);
  gate prob applied as per-partition scalar on ScalarE, experts accumulated
  on VectorE into acc, pre-initialized with sum_e gate_e * bias4_e via a
  PE-transposed-gates matmul.  For the last expert, L4 of the first batch
  half is interleaved between the two L3 halves and the final output DMAs
  alternate between both queues, shortening the drain tail.
"""

import numpy as np
import ml_dtypes
from contextlib import ExitStack

import concourse.bass as bass
import concourse.mybir as mybir
import concourse.tile as tile
from concourse import bacc
from concourse.bass import ts
from concourse.bass_utils import run_bass_kernel_spmd

B, D, E, H, O = 8192, 1024, 12, 512, 256
NCORES = 8
BL = B // NCORES          # 1024 batch rows per core
EPS = 1e-5
F32 = mybir.dt.float32
F32R = mybir.dt.float32r
BF16 = mybir.dt.bfloat16
AF = mybir.ActivationFunctionType
ALU = mybir.AluOpType
AX = mybir.AxisListType
NPBF16 = ml_dtypes.bfloat16

DCH = D // 128            # 8  d-chunks
H1CH = H // 128           # 4  h1-chunks
H3CH = (H // 2) // 128    # 2  h3-chunks
BCH = BL // 128           # 8  b-chunks of 128
BH = BL // 512            # 2  b-halves of 512
NB = 512                  # moving free dim for layers 1-3
NFILL = 2                 # fp32 filler matmuls before the first x chunk lands


def _build_bass():
    nc = bacc.Bacc("TRN2", target_bir_lowering=False, debug=False,
                   enable_asserts=False, num_devices=NCORES)

    xt_d = nc.dram_tensor("xt", [DCH, 128, BL], BF16, kind="ExternalInput")
    w1_d = nc.dram_tensor("w1", [E, 128, DCH, H], BF16, kind="ExternalInput")
    w2_d = nc.dram_tensor("w2", [E, 128, H1CH, H], BF16, kind="ExternalInput")
    w3_d = nc.dram_tensor("w3", [E, 128, H1CH, H // 2], BF16, kind="ExternalInput")
    w4_d = nc.dram_tensor("w4", [E, 128, H3CH, O], BF16, kind="ExternalInput")
    eb_d = nc.dram_tensor("eb", [128, E, 10], F32, kind="ExternalInput")
    # packed small constants:
    #   gwb (bf16) cols: [0:2048 gw1 (dc-major) | 2048:2304 gw2]
    #   pkr (f32r) cols: [0:128 ones | 128:140 gw3 | 140:152 gb3 | 152:408 bmat]
    #   pkf (f32)  cols: [0:2 gb1 | 2:3 gb2 | 3:131 ident]
    gwb_d = nc.dram_tensor("gwb", [128, 2304], BF16, kind="ExternalInput")
    pkr_d = nc.dram_tensor("pkr", [128, 408], F32R, kind="ExternalInput")
    pkf_d = nc.dram_tensor("pkf", [128, 131], F32, kind="ExternalInput")
    out_d = nc.dram_tensor("out", [BL, O], F32, kind="ExternalOutput")

    with tile.TileContext(nc) as tc, ExitStack() as ctx:
        const = ctx.enter_context(tc.tile_pool(name="const", bufs=1))
        gatep = ctx.enter_context(tc.tile_pool(name="gatep", bufs=1))
        gtmp = ctx.enter_context(tc.tile_pool(name="gtmp", bufs=2))
        wpool = ctx.enter_context(tc.tile_pool(name="wpool", bufs=2))
        actp = ctx.enter_context(tc.tile_pool(name="actp", bufs=1))
        accp = ctx.enter_context(tc.tile_pool(name="accp", bufs=1))
        tmpp = ctx.enter_context(tc.tile_pool(name="tmpp", bufs=4))
        # single 8-bank PSUM ring (every slot sized [128, 512] fp32 = 1 bank)
        psP = ctx.enter_context(tc.tile_pool(name="psP", bufs=8, space="PSUM"))

        # ---- startup DMAs in strict consumption order on two queues ----
        # Each dc's (x[dc], w1[dc]) pair lands on opposite queues so both
        # streams hit the per-chunk milestones in lockstep.
        scr = const.tile([128, 512], F32)
        nc.vector.memset(scr, 0.0)
        xtg = const.tile([128, DCH, BL], BF16)
        w1t0 = wpool.tile([128, DCH, H], BF16, name="w1t")
        for dc in range(DCH):
            qx = nc.sync if dc % 2 == 0 else nc.gpsimd
            qw = nc.gpsimd if dc % 2 == 0 else nc.sync
            qx.dma_start(out=xtg[:, dc], in_=xt_d.ap()[dc])
            qw.dma_start(out=w1t0[:, dc], in_=w1_d.ap()[0, :, dc])
        pkf = const.tile([128, 131], F32)
        nc.sync.dma_start(out=pkf, in_=pkf_d.ap())
        ebt = const.tile([128, E, 10], F32)
        nc.gpsimd.dma_start(out=ebt, in_=eb_d.ap())
        gwb = const.tile([128, 2304], BF16)
        nc.gpsimd.dma_start(out=gwb, in_=gwb_d.ap())
        w2t0 = wpool.tile([128, H1CH, H], BF16, name="w2t")
        nc.sync.dma_start(out=w2t0, in_=w2_d.ap()[0])
        pkr = const.tile([128, 408], F32R)
        nc.sync.dma_start(out=pkr, in_=pkr_d.ap())
        w3t0 = wpool.tile([128, H1CH, H // 2], BF16, name="w3t")
        nc.gpsimd.dma_start(out=w3t0, in_=w3_d.ap()[0])
        w4t0 = wpool.tile([128, H3CH, O], BF16, name="w4t")
        nc.gpsimd.dma_start(out=w4t0, in_=w4_d.ap()[0])
        wts = (w1t0, w2t0, w3t0, w4t0)

        def wtiles(e, q1, q2):
            w1t = wpool.tile([128, DCH, H], BF16, name="w1t")
            q1.dma_start(out=w1t[:, :DCH // 2], in_=w1_d.ap()[e, :, :DCH // 2])
            q2.dma_start(out=w1t[:, DCH // 2:], in_=w1_d.ap()[e, :, DCH // 2:])
            w2t = wpool.tile([128, H1CH, H], BF16, name="w2t")
            q1.dma_start(out=w2t, in_=w2_d.ap()[e])
            w3t = wpool.tile([128, H1CH, H // 2], BF16, name="w3t")
            q2.dma_start(out=w3t, in_=w3_d.ap()[e])
            w4t = wpool.tile([128, H3CH, O], BF16, name="w4t")
            q1.dma_start(out=w4t, in_=w4_d.ap()[e])
            return w1t, w2t, w3t, w4t

        gw1v = gwb[:, 0:2048].rearrange("p (c m) -> p c m", c=DCH)
        gw2v = gwb[:, 2048:2304].rearrange("p (c m) -> p c m", c=2)
        ones = pkr[:1, 0:128]
        gw3 = pkr[:, 128:140]
        gb3 = pkr[:1, 140:152]
        bmat = pkr[:E, 152:408]
        gb1 = pkf[:, 0:2]
        gb2 = pkf[:, 2:3]
        ident = pkf[:, 3:131]
        acc = accp.tile([128, BCH, O], F32)

        h1t = actp.tile([128, H1CH, BL], BF16, name="h1t")

        # ---- PE filler: keep the HAM clock-gate released until x[0] lands ----
        for _ in range(NFILL):
            wps = psP.tile([128, 512], F32, name="fill", tag="ps")
            nc.tensor.matmul(wps[:, :512], scr[:, :128], scr, start=True, stop=True)

        # ---- expert-0 layer 1, dc-streaming: 8 open PSUM groups consume
        #      (x[dc], w1[dc]) chunk pairs as the DMAs land ----
        ps_l1 = [psP.tile([128, NB], F32, name=f"l1g{g}", tag="ps") for g in range(8)]
        for dc in range(DCH):
            for bh in range(BH):
                for hc in range(H1CH):
                    nc.tensor.matmul(ps_l1[bh * H1CH + hc],
                                     w1t0[:, dc, ts(hc, 128)],
                                     xtg[:, dc, ts(bh, NB)],
                                     start=(dc == 0), stop=(dc == DCH - 1))
        for bh in range(BH):
            for hc in range(H1CH):
                nc.vector.tensor_scalar(h1t[:, hc, ts(bh, NB)],
                                        ps_l1[bh * H1CH + hc],
                                        ebt[:, 0, hc:hc + 1], 0.0,
                                        ALU.add, ALU.max)

        # ---- gate network (x fully resident now; runs dense) ----
        g1t = gatep.tile([128, 2, BL], BF16)
        g2t = gatep.tile([128, BL], F32R)
        gates = gatep.tile([128, BCH, E], F32)
        for bh in range(BH):
            for hc in range(2):
                ps = psP.tile([128, NB], F32, tag="ps")
                for dc in range(DCH):
                    nc.tensor.matmul(ps, gw1v[:, dc, ts(hc, 128)],
                                     xtg[:, dc, ts(bh, NB)],
                                     start=(dc == 0), stop=(dc == DCH - 1))
                nc.scalar.activation(g1t[:, hc, ts(bh, NB)], ps, AF.Relu,
                                     bias=gb1[:, hc:hc + 1])
            ps = psP.tile([128, NB], F32, tag="ps")
            for kc in range(2):
                nc.tensor.matmul(ps, gw2v[:, kc, :], g1t[:, kc, ts(bh, NB)],
                                 start=(kc == 0), stop=(kc == 1))
            nc.scalar.activation(g2t[:, ts(bh, NB)], ps, AF.Relu, bias=gb2[:, 0:1])
        psgall = psP.tile([128, BCH, E], F32, name="psgall", tag="ps")
        for bc in range(BCH):
            nc.tensor.matmul(psgall[:, bc, :], g2t[:, ts(bc, 128)], gw3,
                             start=True, stop=False)
            nc.tensor.matmul(psgall[:, bc, :], ones[:1, :], gb3[:1, :],
                             start=False, stop=True)
        exall = gatep.tile([128, BCH, E], F32)
        nc.scalar.activation(exall, psgall, AF.Exp)
        sms = gtmp.tile([128, BCH], F32)
        nc.vector.tensor_reduce(sms, exall, AX.X, ALU.add)
        rcs = gtmp.tile([128, BCH], F32)
        nc.vector.reciprocal(rcs, sms)
        for bc in range(BCH):
            nc.scalar.activation(gates[:, bc, :], exall[:, bc, :], AF.Copy,
                                 scale=rcs[:, bc:bc + 1])

        # ---- init acc with the gate-weighted layer-4 bias: acc = gates @ B ----
        gTall = gatep.tile([E, BCH, 128], F32R)
        for bc in range(BCH):
            gps = psP.tile([E, 128], F32, name="gps", tag="ps")
            nc.tensor.transpose(gps, gates[:, bc, :], ident)
            nc.scalar.activation(gTall[:, bc, :], gps, AF.Copy)
        for bc in range(BCH):
            bps = psP.tile([128, O], F32, name="bps", tag="ps")
            nc.tensor.matmul(bps, gTall[:, bc, :], bmat, start=True, stop=True)
            nc.vector.tensor_copy(acc[:, bc, :], bps)

        # ---- experts (weights software-pipelined one expert ahead) ----
        def layer4(e, w4t, h3t, bcs):
            for bc in bcs:
                p4 = psP.tile([128, O], F32, name="p4", tag="ps")
                nc.tensor.matmul(p4, h3t[:, 0, ts(bc, 128)], w4t[:, 0, :],
                                 start=True, stop=False)
                nc.tensor.matmul(p4, h3t[:, 1, ts(bc, 128)], w4t[:, 1, :],
                                 start=False, stop=True)
                tm = tmpp.tile([128, O], F32)
                nc.scalar.activation(tm, p4, AF.Copy, scale=gates[:, bc, e:e + 1])
                nc.vector.tensor_add(acc[:, bc, :], acc[:, bc, :], tm)
                if e == E - 1:
                    q = nc.sync if bc % 2 == 0 else nc.gpsimd
                    q.dma_start(out=out_d.ap()[ts(bc, 128), :], in_=acc[:, bc, :])

        for e in range(E):
            w1t, w2t, w3t, w4t = wts
            if e + 1 < E:
                wts = wtiles(e + 1, nc.sync, nc.gpsimd)

            if e > 0:
                h1t = actp.tile([128, H1CH, BL], BF16, name="h1t")
                for bh in range(BH):        # layer 1: [1024] -> [512]
                    for hc in range(H1CH):
                        ps = psP.tile([128, NB], F32, tag="ps")
                        for dc in range(DCH):
                            nc.tensor.matmul(ps, w1t[:, dc, ts(hc, 128)],
                                             xtg[:, dc, ts(bh, NB)],
                                             start=(dc == 0), stop=(dc == DCH - 1))
                        nc.vector.tensor_scalar(h1t[:, hc, ts(bh, NB)], ps,
                                                ebt[:, e, hc:hc + 1], 0.0,
                                                ALU.add, ALU.max)
            h2t = actp.tile([128, H1CH, BL], BF16, name="h2t")
            h3t = actp.tile([128, H3CH, BL], BF16, name="h3t")
            for bh in range(BH):            # layer 2: [512] -> [512]
                for hc in range(H1CH):
                    ps = psP.tile([128, NB], F32, tag="ps")
                    for kc in range(H1CH):
                        nc.tensor.matmul(ps, w2t[:, kc, ts(hc, 128)], h1t[:, kc, ts(bh, NB)],
                                         start=(kc == 0), stop=(kc == H1CH - 1))
                    nc.scalar.activation(h2t[:, hc, ts(bh, NB)], ps, AF.Relu,
                                         bias=ebt[:, e, 4 + hc:5 + hc])
            for bh in range(BH):            # layer 3: [512] -> [256]
                for hc in range(H3CH):
                    ps = psP.tile([128, NB], F32, tag="ps")
                    for kc in range(H1CH):
                        nc.tensor.matmul(ps, w3t[:, kc, ts(hc, 128)], h2t[:, kc, ts(bh, NB)],
                                         start=(kc == 0), stop=(kc == H1CH - 1))
                    nc.scalar.activation(h3t[:, hc, ts(bh, NB)], ps, AF.Relu,
                                         bias=ebt[:, e, 8 + hc:9 + hc])
                if e == E - 1 and bh == 0:  # drain first-half L4 early
                    layer4(e, w4t, h3t, range(BCH // 2))
            if e == E - 1:
                layer4(e, w4t, h3t, range(BCH // 2, BCH))
            else:
                layer4(e, w4t, h3t, range(BCH))

    nc.compile()
    return nc


def _tile128(w):
    """[K, N] -> [128, K//128, N] with per-partition-contiguous bytes."""
    k, n = w.shape
    return np.ascontiguousarray(w.reshape(k // 128, 128, n).transpose(1, 0, 2))


def _fold(inputs):
    """Fold BatchNorms into next-layer weights/biases (float64 for exactness)."""
    f = {k: np.asarray(v, dtype=np.float64) for k, v in inputs.items()}

    def sb(g, b, m, v):
        s = g / np.sqrt(v + EPS)
        return s, b - m * s

    out = {}
    # gate
    sg1, tg1 = sb(f["gbn1_g"], f["gbn1_b"], f["gbn1_m"], f["gbn1_v"])
    sg2, tg2 = sb(f["gbn2_g"], f["gbn2_b"], f["gbn2_m"], f["gbn2_v"])
    gw1t = _tile128(f["gw1"])                     # [128, DCH, 256]
    gb1c = f["gb1"]
    gw2t = _tile128(sg1[:, None] * f["gw2"])      # [128, 2, 128]
    gb2c = f["gb2"] + tg1 @ f["gw2"]
    gw3t = sg2[:, None] * f["gw3"]                # [128, E]
    gb3r = f["gb3"] + tg2 @ f["gw3"]
    # experts
    s1, t1 = sb(f["ebn1_g"], f["ebn1_b"], f["ebn1_m"], f["ebn1_v"])   # [E,H]
    s2, t2 = sb(f["ebn2_g"], f["ebn2_b"], f["ebn2_m"], f["ebn2_v"])   # [E,H]
    s3, t3 = sb(f["ebn3_g"], f["ebn3_b"], f["ebn3_m"], f["ebn3_v"])   # [E,H/2]
    bf = {}
    bf["w1"] = np.stack([_tile128(f["ew1"][e]) for e in range(E)])
    b1 = f["eb1"]                                                     # [E,H]
    bf["w2"] = np.stack([_tile128(s1[e][:, None] * f["ew2"][e]) for e in range(E)])
    b2 = f["eb2"] + np.einsum("eh,eho->eo", t1, f["ew2"])
    bf["w3"] = np.stack([_tile128(s2[e][:, None] * f["ew3"][e]) for e in range(E)])
    b3 = f["eb3"] + np.einsum("eh,eho->eo", t2, f["ew3"])
    bf["w4"] = np.stack([_tile128(s3[e][:, None] * f["ew4"][e]) for e in range(E)])
    b4 = f["eb4"] + np.einsum("eh,eho->eo", t3, f["ew4"])
    # packed activation-bias columns: [E, 128, 10]
    eb = np.zeros((E, 128, 10))
    eb[:, :, 0:4] = b1.reshape(E, 4, 128).transpose(0, 2, 1)
    eb[:, :, 4:8] = b2.reshape(E, 4, 128).transpose(0, 2, 1)
    eb[:, :, 8:10] = b3.reshape(E, 2, 128).transpose(0, 2, 1)
    out["eb"] = eb.transpose(1, 0, 2)             # [128, E, 10]
    gwb = np.zeros((128, 2304))
    gwb[:, 0:2048] = gw1t.reshape(128, 2048)
    gwb[:, 2048:2304] = gw2t.reshape(128, 256)
    bf["gwb"] = gwb
    pkr = np.zeros((128, 408))
    pkr[:1, 0:128] = 1.0                          # ones row
    pkr[:, 128:140] = gw3t
    pkr[:1, 140:152] = gb3r
    pkr[:E, 152:408] = b4
    out["pkr"] = pkr
    pkf = np.zeros((128, 131))
    pkf[:, 0:2] = gb1c.reshape(2, 128).T
    pkf[:, 2:3] = gb2c.reshape(1, 128).T
    pkf[:, 3:131] = np.eye(128)
    out["pkf"] = pkf
    out = {k: np.ascontiguousarray(v, dtype=np.float32) for k, v in out.items()}
    for k, v in bf.items():
        out[k] = np.ascontiguousarray(v, dtype=NPBF16)
    return out


_CACHE = {}


def build_in_maps(inputs):
    w = _fold(inputs)
    xt_full = np.asarray(inputs["x"], dtype=np.float32).T               # [D, B]
    in_maps = []
    for c in range(NCORES):
        m = dict(w)
        m["xt"] = np.ascontiguousarray(
            xt_full[:, c * BL:(c + 1) * BL].reshape(DCH, 128, BL).astype(NPBF16))
        in_maps.append(m)

    return in_maps


def kernel(**inputs) -> np.ndarray:
    if "nc" not in _CACHE:
        _CACHE["nc"] = _build_bass()
    nc = _CACHE["nc"]

    in_maps = build_in_maps(inputs)
    res = run_bass_kernel_spmd(nc, in_maps, core_ids=list(range(NCORES)))
    return np.concatenate([r["out"] for r in res.results], axis=0)


# revision 4
# speedup vs baseline: 1.0900x; 1.0900x over previous
"""Trainium2 Bass kernel for nn_MixtureOfExperts (B=8192, D=1024, E=12, H=512, O=256).

Strategy:
- Data-parallel over 8 NeuronCores: each core processes 1024 rows of x; all
  weights replicated. Host gathers/concats core outputs.
- Host-side prep: eval-mode BatchNorm (which follows each ReLU) is folded into
  the NEXT layer's weights and bias:  bn(relu(z)) = s*relu(z) + t  with
  s = g/sqrt(v+eps) > 0, t = b - m*s, so
      bn(relu(z)) @ W + c  ==  relu(z) @ (diag(s) W) + (c + t @ W).
  x is pre-transposed and all weights pre-tiled on host into the exact SBUF
  layout ([128 part, chunk, free] with per-partition-contiguous DRAM bytes) so
  every big DMA is a fully contiguous copy.
- All matmuls fp32r (full PE rate with moving free >= 256; measured faster
  than bf16 on this part: bf16 N=512 matmuls issue at ~250ns vs ~228ns fp32r).
- Startup is DMA-bound, so the PE is fed with real work in DMA-arrival order:
  expert-0 L1 runs FIRST in dc-streaming order with 8 open PSUM accumulation
  groups, consuming (x[dc], w1[dc]) chunk pairs as they land (each pair split
  across the two DMA queues).  Two fp32 filler matmuls cover the
  pre-first-chunk gap so the HAM clock-gate stays released.  The gate g1 then
  runs dc-streamed off per-chunk gwb slices, and the gate tail (g2/g3/softmax/
  bias-init) is interleaved between expert-0's L2/L3/L4 so the PE never waits
  on the softmax ACT/DVE chain.
- Layers 1-3 feature-major; layer 4 batch-major (stationary = h3T slice);
  gate prob applied as per-partition scalar on ScalarE, experts accumulated
  on VectorE into acc, pre-initialized with sum_e gate_e * bias4_e via a
  PE-transposed-gates matmul.  For the last expert, L4 of the first batch
  half is interleaved between the two L3 halves and the final output DMAs
  alternate between both queues, shortening the drain tail.
"""

import numpy as np
import ml_dtypes
from contextlib import ExitStack

import concourse.bass as bass
import concourse.mybir as mybir
import concourse.tile as tile
from concourse import bacc
from concourse.bass import ts
from concourse.bass_utils import run_bass_kernel_spmd

B, D, E, H, O = 8192, 1024, 12, 512, 256
NCORES = 8
BL = B // NCORES          # 1024 batch rows per core
EPS = 1e-5
F32 = mybir.dt.float32
F32R = mybir.dt.float32r
BF16 = mybir.dt.bfloat16
AF = mybir.ActivationFunctionType
ALU = mybir.AluOpType
AX = mybir.AxisListType
NPBF16 = ml_dtypes.bfloat16

DCH = D // 128            # 8  d-chunks
H1CH = H // 128           # 4  h1-chunks
H3CH = (H // 2) // 128    # 2  h3-chunks
BCH = BL // 128           # 8  b-chunks of 128
BH = BL // 512            # 2  b-halves of 512
NB = 512                  # moving free dim for layers 1-3
NFILL = 2                 # fp32 filler matmuls before the first x chunk lands


def _build_bass():
    nc = bacc.Bacc("TRN2", target_bir_lowering=False, debug=False,
                   enable_asserts=False, num_devices=NCORES)

    xt_d = nc.dram_tensor("xt", [DCH, 128, BL], F32R, kind="ExternalInput")
    w1_d = nc.dram_tensor("w1", [E, 128, DCH, H], F32R, kind="ExternalInput")
    w2_d = nc.dram_tensor("w2", [E, 128, H1CH, H], F32R, kind="ExternalInput")
    w3_d = nc.dram_tensor("w3", [E, 128, H1CH, H // 2], F32R, kind="ExternalInput")
    w4_d = nc.dram_tensor("w4", [E, 128, H3CH, O], F32R, kind="ExternalInput")
    eb_d = nc.dram_tensor("eb", [128, E, 10], F32, kind="ExternalInput")
    # packed small constants:
    #   gwb (f32r) cols: [0:2048 gw1 (dc-major) | 2048:2304 gw2]
    #   pkr (f32r) cols: [0:128 ones | 128:140 gw3 | 140:152 gb3 | 152:408 bmat]
    #   pkf (f32)  cols: [0:2 gb1 | 2:3 gb2 | 3:131 ident]
    gwb_d = nc.dram_tensor("gwb", [128, 2304], F32R, kind="ExternalInput")
    pkr_d = nc.dram_tensor("pkr", [128, 408], F32R, kind="ExternalInput")
    pkf_d = nc.dram_tensor("pkf", [128, 131], F32, kind="ExternalInput")
    out_d = nc.dram_tensor("out", [BL, O], F32, kind="ExternalOutput")

    with tile.TileContext(nc) as tc, ExitStack() as ctx:
        const = ctx.enter_context(tc.tile_pool(name="const", bufs=1))
        gatep = ctx.enter_context(tc.tile_pool(name="gatep", bufs=1))
        gtmp = ctx.enter_context(tc.tile_pool(name="gtmp", bufs=2))
        wpool = ctx.enter_context(tc.tile_pool(name="wpool", bufs=2))
        actp = ctx.enter_context(tc.tile_pool(name="actp", bufs=1))
        accp = ctx.enter_context(tc.tile_pool(name="accp", bufs=1))
        tmpp = ctx.enter_context(tc.tile_pool(name="tmpp", bufs=4))
        # single 8-bank PSUM ring (every slot sized [128, 512] fp32 = 1 bank)
        psP = ctx.enter_context(tc.tile_pool(name="psP", bufs=8, space="PSUM"))

        # ---- startup DMAs in strict consumption order on two queues ----
        scr = const.tile([128, 512], F32)
        nc.vector.memset(scr, 0.0)
        pkf = const.tile([128, 131], F32)
        nc.sync.dma_start(out=pkf, in_=pkf_d.ap())
        ebt = const.tile([128, E, 10], F32)
        nc.gpsimd.dma_start(out=ebt, in_=eb_d.ap())
        # (x[dc], w1[dc]) pairs on opposite queues so both streams hit the
        # per-chunk milestones in lockstep
        xtg = const.tile([128, DCH, BL], F32R)
        w1t0 = wpool.tile([128, DCH, H], F32R, name="w1t")
        for dc in range(DCH):
            qx = nc.sync if dc % 2 == 0 else nc.gpsimd
            qw = nc.gpsimd if dc % 2 == 0 else nc.sync
            qx.dma_start(out=xtg[:, dc], in_=xt_d.ap()[dc])
            qw.dma_start(out=w1t0[:, dc], in_=w1_d.ap()[0, :, dc])
        # gwb per-dc slices so gate g1 can start on slice 0
        gwb = const.tile([128, 2304], F32R)
        for dc in range(DCH):
            eng = nc.sync if dc % 2 == 0 else nc.gpsimd
            eng.dma_start(out=gwb[:, ts(dc, 256)], in_=gwb_d.ap()[:, ts(dc, 256)])
        nc.sync.dma_start(out=gwb[:, 2048:2304], in_=gwb_d.ap()[:, 2048:2304])
        w2t0 = wpool.tile([128, H1CH, H], F32R, name="w2t")
        nc.gpsimd.dma_start(out=w2t0[:, :2], in_=w2_d.ap()[0, :, :2])
        nc.sync.dma_start(out=w2t0[:, 2:], in_=w2_d.ap()[0, :, 2:])
        pkr = const.tile([128, 408], F32R)
        nc.gpsimd.dma_start(out=pkr, in_=pkr_d.ap())
        w3t0 = wpool.tile([128, H1CH, H // 2], F32R, name="w3t")
        nc.sync.dma_start(out=w3t0[:, :2], in_=w3_d.ap()[0, :, :2])
        nc.gpsimd.dma_start(out=w3t0[:, 2:], in_=w3_d.ap()[0, :, 2:])
        w4t0 = wpool.tile([128, H3CH, O], F32R, name="w4t")
        nc.sync.dma_start(out=w4t0, in_=w4_d.ap()[0])

        def wtiles(e, q1, q2):
            w1t = wpool.tile([128, DCH, H], F32R, name="w1t")
            q1.dma_start(out=w1t[:, :DCH // 2], in_=w1_d.ap()[e, :, :DCH // 2])
            q2.dma_start(out=w1t[:, DCH // 2:], in_=w1_d.ap()[e, :, DCH // 2:])
            w2t = wpool.tile([128, H1CH, H], F32R, name="w2t")
            q1.dma_start(out=w2t, in_=w2_d.ap()[e])
            w3t = wpool.tile([128, H1CH, H // 2], F32R, name="w3t")
            q2.dma_start(out=w3t, in_=w3_d.ap()[e])
            w4t = wpool.tile([128, H3CH, O], F32R, name="w4t")
            q1.dma_start(out=w4t, in_=w4_d.ap()[e])
            return w1t, w2t, w3t, w4t

        gw1v = gwb[:, 0:2048].rearrange("p (c m) -> p c m", c=DCH)
        gw2v = gwb[:, 2048:2304].rearrange("p (c m) -> p c m", c=2)
        ones = pkr[:1, 0:128]
        gw3 = pkr[:, 128:140]
        gb3 = pkr[:1, 140:152]
        bmat = pkr[:E, 152:408]
        gb1 = pkf[:, 0:2]
        gb2 = pkf[:, 2:3]
        ident = pkf[:, 3:131]
        acc = accp.tile([128, BCH, O], F32)

        # ---- PE filler: keep the HAM clock-gate released until x[0] lands ----
        for _ in range(NFILL):
            wps = psP.tile([128, 512], F32, name="fill", tag="ps")
            nc.tensor.matmul(wps, scr[:, :128], scr, start=True, stop=True)

        # ---- expert-0 layer 1, dc-streaming: 8 open PSUM groups consume
        #      (x[dc], w1[dc]) chunk pairs as the DMAs land ----
        h1t = actp.tile([128, H1CH, BL], F32R, name="h1t")
        ps_l1 = [psP.tile([128, NB], F32, name=f"l1g{g}", tag="ps") for g in range(8)]
        for dc in range(DCH):
            for bh in range(BH):
                for hc in range(H1CH):
                    nc.tensor.matmul(ps_l1[bh * H1CH + hc],
                                     w1t0[:, dc, ts(hc, 128)],
                                     xtg[:, dc, ts(bh, NB)],
                                     start=(dc == 0), stop=(dc == DCH - 1))
        for bh in range(BH):
            for hc in range(H1CH):
                nc.vector.tensor_scalar(h1t[:, hc, ts(bh, NB)],
                                        ps_l1[bh * H1CH + hc],
                                        ebt[:, 0, hc:hc + 1], 0.0,
                                        ALU.add, ALU.max)

        # ---- gate g1, dc-streaming off the gwb slices (4 open groups) ----
        g1t = gatep.tile([128, 2, BL], F32R)
        g2t = gatep.tile([128, BL], F32R)
        gates = gatep.tile([128, BCH, E], F32)
        ps_g1 = [psP.tile([128, NB], F32, name=f"g1g{g}", tag="ps") for g in range(4)]
        for dc in range(DCH):
            for bh in range(BH):
                for hc in range(2):
                    nc.tensor.matmul(ps_g1[bh * 2 + hc],
                                     gw1v[:, dc, ts(hc, 128)],
                                     xtg[:, dc, ts(bh, NB)],
                                     start=(dc == 0), stop=(dc == DCH - 1))
        for bh in range(BH):
            for hc in range(2):
                nc.scalar.activation(g1t[:, hc, ts(bh, NB)], ps_g1[bh * 2 + hc],
                                     AF.Relu, bias=gb1[:, hc:hc + 1])

        # prefetch expert-1 weights behind the startup stream
        wts = wtiles(1, nc.sync, nc.gpsimd)

        # ---- expert-0 layer 2 (dense; overlaps gate-tail ACT/DVE work) ----
        h2t = actp.tile([128, H1CH, BL], F32R, name="h2t")
        h3t = actp.tile([128, H3CH, BL], F32R, name="h3t")
        for bh in range(BH):
            for hc in range(H1CH):
                ps = psP.tile([128, NB], F32, tag="ps")
                for kc in range(H1CH):
                    nc.tensor.matmul(ps, w2t0[:, kc, ts(hc, 128)], h1t[:, kc, ts(bh, NB)],
                                     start=(kc == 0), stop=(kc == H1CH - 1))
                nc.scalar.activation(h2t[:, hc, ts(bh, NB)], ps, AF.Relu,
                                     bias=ebt[:, 0, 4 + hc:5 + hc])

        # ---- gate g2/g3 + softmax ----
        for bh in range(BH):
            ps = psP.tile([128, NB], F32, tag="ps")
            for kc in range(2):
                nc.tensor.matmul(ps, gw2v[:, kc, :], g1t[:, kc, ts(bh, NB)],
                                 start=(kc == 0), stop=(kc == 1))
            nc.scalar.activation(g2t[:, ts(bh, NB)], ps, AF.Relu, bias=gb2[:, 0:1])
        psgall = psP.tile([128, BCH, E], F32, name="psgall", tag="ps")
        for bc in range(BCH):
            nc.tensor.matmul(psgall[:, bc, :], g2t[:, ts(bc, 128)], gw3,
                             start=True, stop=False)
            nc.tensor.matmul(psgall[:, bc, :], ones[:1, :], gb3[:1, :],
                             start=False, stop=True)
        exall = gatep.tile([128, BCH, E], F32)
        nc.scalar.activation(exall, psgall, AF.Exp)
        sms = gtmp.tile([128, BCH], F32)
        nc.vector.tensor_reduce(sms, exall, AX.X, ALU.add)
        rcs = gtmp.tile([128, BCH], F32)
        nc.vector.reciprocal(rcs, sms)
        for bc in range(BCH):
            nc.scalar.activation(gates[:, bc, :], exall[:, bc, :], AF.Copy,
                                 scale=rcs[:, bc:bc + 1])

        # ---- expert-0 layer 3 (the softmax chain drains meanwhile) ----
        for bh in range(BH):
            for hc in range(H3CH):
                ps = psP.tile([128, NB], F32, tag="ps")
                for kc in range(H1CH):
                    nc.tensor.matmul(ps, w3t0[:, kc, ts(hc, 128)], h2t[:, kc, ts(bh, NB)],
                                     start=(kc == 0), stop=(kc == H1CH - 1))
                nc.scalar.activation(h3t[:, hc, ts(bh, NB)], ps, AF.Relu,
                                     bias=ebt[:, 0, 8 + hc:9 + hc])

        # ---- init acc with the gate-weighted layer-4 bias: acc = gates @ B ----
        gTall = gatep.tile([E, BCH, 128], F32R)
        for bc in range(BCH):
            gps = psP.tile([E, 128], F32, name="gps", tag="ps")
            nc.tensor.transpose(gps, gates[:, bc, :], ident)
            nc.scalar.activation(gTall[:, bc, :], gps, AF.Copy)
        for bc in range(BCH):
            bps = psP.tile([128, O], F32, name="bps", tag="ps")
            nc.tensor.matmul(bps, gTall[:, bc, :], bmat, start=True, stop=True)
            nc.vector.tensor_copy(acc[:, bc, :], bps)

        def layer4(e, w4t, h3t, bcs):
            for bc in bcs:
                p4 = psP.tile([128, O], F32, name="p4", tag="ps")
                nc.tensor.matmul(p4, h3t[:, 0, ts(bc, 128)], w4t[:, 0, :],
                                 start=True, stop=False)
                nc.tensor.matmul(p4, h3t[:, 1, ts(bc, 128)], w4t[:, 1, :],
                                 start=False, stop=True)
                tm = tmpp.tile([128, O], F32)
                nc.scalar.activation(tm, p4, AF.Copy, scale=gates[:, bc, e:e + 1])
                nc.vector.tensor_add(acc[:, bc, :], acc[:, bc, :], tm)
                if e == E - 1:
                    q = nc.sync if bc % 2 == 0 else nc.gpsimd
                    q.dma_start(out=out_d.ap()[ts(bc, 128), :], in_=acc[:, bc, :])

        # ---- expert-0 layer 4 ----
        layer4(0, w4t0, h3t, range(BCH))

        # ---- experts 1..11 (weights software-pipelined one expert ahead) ----
        for e in range(1, E):
            w1t, w2t, w3t, w4t = wts
            if e + 1 < E:
                wts = wtiles(e + 1, nc.sync, nc.gpsimd)

            h1t = actp.tile([128, H1CH, BL], F32R, name="h1t")
            for bh in range(BH):            # layer 1: [1024] -> [512]
                for hc in range(H1CH):
                    ps = psP.tile([128, NB], F32, tag="ps")
                    for dc in range(DCH):
                        nc.tensor.matmul(ps, w1t[:, dc, ts(hc, 128)],
                                         xtg[:, dc, ts(bh, NB)],
                                         start=(dc == 0), stop=(dc == DCH - 1))
                    nc.vector.tensor_scalar(h1t[:, hc, ts(bh, NB)], ps,
                                            ebt[:, e, hc:hc + 1], 0.0,
                                            ALU.add, ALU.max)
            h2t = actp.tile([128, H1CH, BL], F32R, name="h2t")
            h3t = actp.tile([128, H3CH, BL], F32R, name="h3t")
            for bh in range(BH):            # layer 2: [512] -> [512]
                for hc in range(H1CH):
                    ps = psP.tile([128, NB], F32, tag="ps")
                    for kc in range(H1CH):
                        nc.tensor.matmul(ps, w2t[:, kc, ts(hc, 128)], h1t[:, kc, ts(bh, NB)],
                                         start=(kc == 0), stop=(kc == H1CH - 1))
                    nc.scalar.activation(h2t[:, hc, ts(bh, NB)], ps, AF.Relu,
                                         bias=ebt[:, e, 4 + hc:5 + hc])
            for bh in range(BH):            # layer 3: [512] -> [256]
                for hc in range(H3CH):
                    ps = psP.tile([128, NB], F32, tag="ps")
                    for kc in range(H1CH):
                        nc.tensor.matmul(ps, w3t[:, kc, ts(hc, 128)], h2t[:, kc, ts(bh, NB)],
                                         start=(kc == 0), stop=(kc == H1CH - 1))
                    nc.scalar.activation(h3t[:, hc, ts(bh, NB)], ps, AF.Relu,
                                         bias=ebt[:, e, 8 + hc:9 + hc])
                if e == E - 1 and bh == 0:  # drain first-half L4 early
                    layer4(e, w4t, h3t, range(BCH // 2))
            if e == E - 1:
                layer4(e, w4t, h3t, range(BCH // 2, BCH))
            else:
                layer4(e, w4t, h3t, range(BCH))

    nc.compile()
    return nc


def _tile128(w):
    """[K, N] -> [128, K//128, N] with per-partition-contiguous bytes."""
    k, n = w.shape
    return np.ascontiguousarray(w.reshape(k // 128, 128, n).transpose(1, 0, 2))


def _fold(inputs):
    """Fold BatchNorms into next-layer weights/biases (float64 for exactness)."""
    f = {k: np.asarray(v, dtype=np.float64) for k, v in inputs.items()}

    def sb(g, b, m, v):
        s = g / np.sqrt(v + EPS)
        return s, b - m * s

    out = {}
    # gate
    sg1, tg1 = sb(f["gbn1_g"], f["gbn1_b"], f["gbn1_m"], f["gbn1_v"])
    sg2, tg2 = sb(f["gbn2_g"], f["gbn2_b"], f["gbn2_m"], f["gbn2_v"])
    gw1t = _tile128(f["gw1"])                     # [128, DCH, 256]
    gb1c = f["gb1"]
    gw2t = _tile128(sg1[:, None] * f["gw2"])      # [128, 2, 128]
    gb2c = f["gb2"] + tg1 @ f["gw2"]
    gw3t = sg2[:, None] * f["gw3"]                # [128, E]
    gb3r = f["gb3"] + tg2 @ f["gw3"]
    # experts
    s1, t1 = sb(f["ebn1_g"], f["ebn1_b"], f["ebn1_m"], f["ebn1_v"])   # [E,H]
    s2, t2 = sb(f["ebn2_g"], f["ebn2_b"], f["ebn2_m"], f["ebn2_v"])   # [E,H]
    s3, t3 = sb(f["ebn3_g"], f["ebn3_b"], f["ebn3_m"], f["ebn3_v"])   # [E,H/2]
    out["w1"] = np.stack([_tile128(f["ew1"][e]) for e in range(E)])
    b1 = f["eb1"]                                                     # [E,H]
    out["w2"] = np.stack([_tile128(s1[e][:, None] * f["ew2"][e]) for e in range(E)])
    b2 = f["eb2"] + np.einsum("eh,eho->eo", t1, f["ew2"])
    out["w3"] = np.stack([_tile128(s2[e][:, None] * f["ew3"][e]) for e in range(E)])
    b3 = f["eb3"] + np.einsum("eh,eho->eo", t2, f["ew3"])
    out["w4"] = np.stack([_tile128(s3[e][:, None] * f["ew4"][e]) for e in range(E)])
    b4 = f["eb4"] + np.einsum("eh,eho->eo", t3, f["ew4"])
    # packed activation-bias columns: [E, 128, 10]
    eb = np.zeros((E, 128, 10))
    eb[:, :, 0:4] = b1.reshape(E, 4, 128).transpose(0, 2, 1)
    eb[:, :, 4:8] = b2.reshape(E, 4, 128).transpose(0, 2, 1)
    eb[:, :, 8:10] = b3.reshape(E, 2, 128).transpose(0, 2, 1)
    out["eb"] = eb.transpose(1, 0, 2)             # [128, E, 10]
    gwb = np.zeros((128, 2304))
    gwb[:, 0:2048] = gw1t.reshape(128, 2048)
    gwb[:, 2048:2304] = gw2t.reshape(128, 256)
    out["gwb"] = gwb
    pkr = np.zeros((128, 408))
    pkr[:1, 0:128] = 1.0                          # ones row
    pkr[:, 128:140] = gw3t
    pkr[:1, 140:152] = gb3r
    pkr[:E, 152:408] = b4
    out["pkr"] = pkr
    pkf = np.zeros((128, 131))
    pkf[:, 0:2] = gb1c.reshape(2, 128).T
    pkf[:, 2:3] = gb2c.reshape(1, 128).T
    pkf[:, 3:131] = np.eye(128)
    out["pkf"] = pkf
    return {k: np.ascontiguousarray(v, dtype=np.float32) for k, v in out.items()}


_CACHE = {}


def build_in_maps(inputs):
    w = _fold(inputs)
    xt_full = np.asarray(inputs["x"], dtype=np.float32).T               # [D, B]
    in_maps = []
    for c in range(NCORES):
        m = dict(w)
        m["xt"] = np.ascontiguousarray(
            xt_full[:, c * BL:(c + 1) * BL].reshape(DCH, 128, BL))
        in_maps.append(m)

    return in_maps


def kernel(**inputs) -> np.ndarray:
    if "nc" not in _CACHE:
        _CACHE["nc"] = _build_bass()
    nc = _CACHE["nc"]

    in_maps = build_in_maps(inputs)
    res = run_bass_kernel_spmd(nc, in_maps, core_ids=list(range(NCORES)))
    return np.concatenate([r["out"] for r in res.results], axis=0)


# revision 11
# speedup vs baseline: 1.1057x; 1.0144x over previous
"""Trainium2 Bass kernel for nn_MixtureOfExperts (B=8192, D=1024, E=12, H=512, O=256).

Strategy:
- Data-parallel over 8 NeuronCores: each core processes 1024 rows of x; all
  weights replicated. Host gathers/concats core outputs.
- Host-side prep: eval-mode BatchNorm (which follows each ReLU) is folded into
  the NEXT layer's weights and bias:  bn(relu(z)) = s*relu(z) + t  with
  s = g/sqrt(v+eps) > 0, t = b - m*s, so
      bn(relu(z)) @ W + c  ==  relu(z) @ (diag(s) W) + (c + t @ W).
  x is pre-transposed and all weights pre-tiled on host into the exact SBUF
  layout ([128 part, chunk, free] with per-partition-contiguous DRAM bytes) so
  every big DMA is a fully contiguous copy.
- All matmuls fp32r (full PE rate with moving free >= 256; measured faster
  than bf16 on this part: bf16 N=512 matmuls issue at ~250ns vs ~228ns fp32r).
- Startup is DMA-bound, so the PE is fed with real work in DMA-arrival order:
  expert-0 L1 runs FIRST in dc-streaming order with 8 open PSUM accumulation
  groups, consuming (x[dc], w1[dc]) chunk pairs as they land (each pair split
  across the two DMA queues).  Two fp32 filler matmuls cover the
  pre-first-chunk gap so the HAM clock-gate stays released.  The gate g1 then
  runs dc-streamed off per-chunk gwb slices, and the gate tail (g2/g3/softmax/
  bias-init) is interleaved between expert-0's L2/L3/L4 so the PE never waits
  on the softmax ACT/DVE chain.
- Layers 1-3 feature-major; layer 4 batch-major (stationary = h3T slice);
  gate prob applied as per-partition scalar on ScalarE, experts accumulated
  on VectorE into acc, pre-initialized with sum_e gate_e * bias4_e via a
  PE-transposed-gates matmul.  For the last expert, L4 of the first batch
  half is interleaved between the two L3 halves and the final output DMAs
  alternate between both queues, shortening the drain tail.
"""

import numpy as np
import ml_dtypes
from contextlib import ExitStack

import concourse.bass as bass
import concourse.mybir as mybir
import concourse.tile as tile
from concourse import bacc
from concourse.bass import ts
from concourse.bass_utils import run_bass_kernel_spmd

B, D, E, H, O = 8192, 1024, 12, 512, 256
NCORES = 8
BL = B // NCORES          # 1024 batch rows per core
EPS = 1e-5
F32 = mybir.dt.float32
F32R = mybir.dt.float32r
BF16 = mybir.dt.bfloat16
AF = mybir.ActivationFunctionType
ALU = mybir.AluOpType
AX = mybir.AxisListType
NPBF16 = ml_dtypes.bfloat16

DCH = D // 128            # 8  d-chunks
H1CH = H // 128           # 4  h1-chunks
H3CH = (H // 2) // 128    # 2  h3-chunks
BCH = BL // 128           # 8  b-chunks of 128
BH = BL // 512            # 2  b-halves of 512
NB = 512                  # moving free dim for layers 1-3
NFILL = 2                 # fp32 filler matmuls before the first x chunk lands


def _build_bass():
    nc = bacc.Bacc("TRN2", target_bir_lowering=False, debug=False,
                   enable_asserts=False, num_devices=NCORES)

    xt_d = nc.dram_tensor("xt", [DCH, 128, BL], F32R, kind="ExternalInput")
    # dc-major duplicate of expert-0's w1 so the startup per-chunk DMAs are
    # fully contiguous (the strided per-chunk reads of w1 throttle the
    # DMA ramp exactly when the PE is starved)
    w1e0_d = nc.dram_tensor("w1e0", [DCH, 128, H], F32R, kind="ExternalInput")
    w1_d = nc.dram_tensor("w1", [E, 128, DCH, H], F32R, kind="ExternalInput")
    w2_d = nc.dram_tensor("w2", [E, 128, H1CH, H], F32R, kind="ExternalInput")
    w3_d = nc.dram_tensor("w3", [E, 128, H1CH, H // 2], F32R, kind="ExternalInput")
    w4_d = nc.dram_tensor("w4", [E, 128, H3CH, O], F32R, kind="ExternalInput")
    eb_d = nc.dram_tensor("eb", [128, E, 10], F32, kind="ExternalInput")
    # packed small constants:
    #   gwb (f32r) cols: [0:2048 gw1 (dc-major) | 2048:2304 gw2]
    #   pkr (f32r) cols: [0:128 ones | 128:140 gw3 | 140:152 gb3 | 152:408 bmat]
    #   pkf (f32)  cols: [0:2 gb1 | 2:3 gb2 | 3:131 ident]
    gwb_d = nc.dram_tensor("gwb", [128, 2304], F32R, kind="ExternalInput")
    pkr_d = nc.dram_tensor("pkr", [128, 408], F32R, kind="ExternalInput")
    pkf_d = nc.dram_tensor("pkf", [128, 131], F32, kind="ExternalInput")
    out_d = nc.dram_tensor("out", [BL, O], F32, kind="ExternalOutput")

    with tile.TileContext(nc) as tc, ExitStack() as ctx:
        const = ctx.enter_context(tc.tile_pool(name="const", bufs=1))
        gatep = ctx.enter_context(tc.tile_pool(name="gatep", bufs=1))
        gtmp = ctx.enter_context(tc.tile_pool(name="gtmp", bufs=2))
        wpool = ctx.enter_context(tc.tile_pool(name="wpool", bufs=2))
        actp = ctx.enter_context(tc.tile_pool(name="actp", bufs=1))
        accp = ctx.enter_context(tc.tile_pool(name="accp", bufs=1))
        tmpp = ctx.enter_context(tc.tile_pool(name="tmpp", bufs=4))
        # single 8-bank PSUM ring (every slot sized [128, 512] fp32 = 1 bank)
        psP = ctx.enter_context(tc.tile_pool(name="psP", bufs=8, space="PSUM"))

        # ---- startup DMAs in strict consumption order on two queues ----
        # Every transfer here reads fully-contiguous DRAM (big packets).
        # gwb goes first (small; fillers cover it) so the gate is never the
        # head-of-line blocker; then (x[dc], w1[dc]) pairs stream on opposite
        # queues so the dc-milestones land in lockstep.
        scr = const.tile([128, 512], F32)
        nc.vector.memset(scr, 0.0)
        gwb = const.tile([128, 2304], F32R)
        nc.sync.dma_start(out=gwb[:, 0:1152], in_=gwb_d.ap()[:, 0:1152])
        nc.gpsimd.dma_start(out=gwb[:, 1152:2304], in_=gwb_d.ap()[:, 1152:2304])
        pkf = const.tile([128, 131], F32)
        nc.sync.dma_start(out=pkf, in_=pkf_d.ap())
        ebt = const.tile([128, E, 10], F32)
        nc.gpsimd.dma_start(out=ebt, in_=eb_d.ap())
        xtg = const.tile([128, DCH, BL], F32R)
        w1t0 = wpool.tile([128, DCH, H], F32R, name="w1t")
        for dc in range(DCH):
            qx = nc.sync if dc % 2 == 0 else nc.gpsimd
            qw = nc.gpsimd if dc % 2 == 0 else nc.sync
            qx.dma_start(out=xtg[:, dc], in_=xt_d.ap()[dc])
            qw.dma_start(out=w1t0[:, dc], in_=w1e0_d.ap()[dc])
        w2t0 = wpool.tile([128, H1CH, H], F32R, name="w2t")
        nc.sync.dma_start(out=w2t0, in_=w2_d.ap()[0])
        w3t0 = wpool.tile([128, H1CH, H // 2], F32R, name="w3t")
        nc.gpsimd.dma_start(out=w3t0, in_=w3_d.ap()[0])
        w4t0 = wpool.tile([128, H3CH, O], F32R, name="w4t")
        nc.gpsimd.dma_start(out=w4t0, in_=w4_d.ap()[0])
        pkr = const.tile([128, 408], F32R)
        nc.sync.dma_start(out=pkr, in_=pkr_d.ap())

        def wtiles(e, q1, q2):
            w1t = wpool.tile([128, DCH, H], F32R, name="w1t")
            q1.dma_start(out=w1t[:, :DCH // 2], in_=w1_d.ap()[e, :, :DCH // 2])
            q2.dma_start(out=w1t[:, DCH // 2:], in_=w1_d.ap()[e, :, DCH // 2:])
            w2t = wpool.tile([128, H1CH, H], F32R, name="w2t")
            q1.dma_start(out=w2t, in_=w2_d.ap()[e])
            w3t = wpool.tile([128, H1CH, H // 2], F32R, name="w3t")
            q2.dma_start(out=w3t, in_=w3_d.ap()[e])
            w4t = wpool.tile([128, H3CH, O], F32R, name="w4t")
            q1.dma_start(out=w4t, in_=w4_d.ap()[e])
            return w1t, w2t, w3t, w4t

        gw1v = gwb[:, 0:2048].rearrange("p (c m) -> p c m", c=DCH)
        gw2v = gwb[:, 2048:2304].rearrange("p (c m) -> p c m", c=2)
        ones = pkr[:1, 0:128]
        gw3 = pkr[:, 128:140]
        gb3 = pkr[:1, 140:152]
        bmat = pkr[:E, 152:408]
        gb1 = pkf[:, 0:2]
        gb2 = pkf[:, 2:3]
        ident = pkf[:, 3:131]
        acc = accp.tile([128, BCH, O], F32)

        # ---- PE filler: keep the HAM clock-gate released until x[0] lands ----
        for _ in range(NFILL):
            wps = psP.tile([128, 512], F32, name="fill", tag="ps")
            nc.tensor.matmul(wps, scr[:, :128], scr, start=True, stop=True)

        # ---- expert-0 layer 1, dc-streaming: 8 open PSUM groups consume
        #      (x[dc], w1[dc]) chunk pairs as the DMAs land ----
        h1t = actp.tile([128, H1CH, BL], F32R, name="h1t")
        ps_l1 = [psP.tile([128, NB], F32, name=f"l1g{g}", tag="ps") for g in range(8)]
        for dc in range(DCH):
            for bh in range(BH):
                for hc in range(H1CH):
                    nc.tensor.matmul(ps_l1[bh * H1CH + hc],
                                     w1t0[:, dc, ts(hc, 128)],
                                     xtg[:, dc, ts(bh, NB)],
                                     start=(dc == 0), stop=(dc == DCH - 1))
        for bh in range(BH):
            for hc in range(H1CH):
                nc.vector.tensor_scalar(h1t[:, hc, ts(bh, NB)],
                                        ps_l1[bh * H1CH + hc],
                                        ebt[:, 0, hc:hc + 1], 0.0,
                                        ALU.add, ALU.max)

        # ---- gate g1, dc-streaming off the gwb slices (4 open groups) ----
        g1t = gatep.tile([128, 2, BL], F32R)
        g2t = gatep.tile([128, BL], F32R)
        gates = gatep.tile([128, BCH, E], F32)
        ps_g1 = [psP.tile([128, NB], F32, name=f"g1g{g}", tag="ps") for g in range(4)]
        for dc in range(DCH):
            for bh in range(BH):
                for hc in range(2):
                    nc.tensor.matmul(ps_g1[bh * 2 + hc],
                                     gw1v[:, dc, ts(hc, 128)],
                                     xtg[:, dc, ts(bh, NB)],
                                     start=(dc == 0), stop=(dc == DCH - 1))
        for bh in range(BH):
            for hc in range(2):
                nc.scalar.activation(g1t[:, hc, ts(bh, NB)], ps_g1[bh * 2 + hc],
                                     AF.Relu, bias=gb1[:, hc:hc + 1])

        # prefetch expert-1 weights behind the startup stream
        wts = wtiles(1, nc.sync, nc.gpsimd)

        # ---- expert-0 layer 2 (dense; overlaps gate-tail ACT/DVE work) ----
        h2t = actp.tile([128, H1CH, BL], F32R, name="h2t")
        h3t = actp.tile([128, H3CH, BL], F32R, name="h3t")
        for bh in range(BH):
            for hc in range(H1CH):
                ps = psP.tile([128, NB], F32, tag="ps")
                for kc in range(H1CH):
                    nc.tensor.matmul(ps, w2t0[:, kc, ts(hc, 128)], h1t[:, kc, ts(bh, NB)],
                                     start=(kc == 0), stop=(kc == H1CH - 1))
                nc.scalar.activation(h2t[:, hc, ts(bh, NB)], ps, AF.Relu,
                                     bias=ebt[:, 0, 4 + hc:5 + hc])

        # ---- gate g2/g3 + softmax ----
        for bh in range(BH):
            ps = psP.tile([128, NB], F32, tag="ps")
            for kc in range(2):
                nc.tensor.matmul(ps, gw2v[:, kc, :], g1t[:, kc, ts(bh, NB)],
                                 start=(kc == 0), stop=(kc == 1))
            nc.scalar.activation(g2t[:, ts(bh, NB)], ps, AF.Relu, bias=gb2[:, 0:1])
        psgall = psP.tile([128, BCH, E], F32, name="psgall", tag="ps")
        for bc in range(BCH):
            nc.tensor.matmul(psgall[:, bc, :], g2t[:, ts(bc, 128)], gw3,
                             start=True, stop=False)
            nc.tensor.matmul(psgall[:, bc, :], ones[:1, :], gb3[:1, :],
                             start=False, stop=True)
        exall = gatep.tile([128, BCH, E], F32)
        nc.scalar.activation(exall, psgall, AF.Exp)
        sms = gtmp.tile([128, BCH], F32)
        nc.vector.tensor_reduce(sms, exall, AX.X, ALU.add)
        rcs = gtmp.tile([128, BCH], F32)
        nc.vector.reciprocal(rcs, sms)
        for bc in range(BCH):
            nc.scalar.activation(gates[:, bc, :], exall[:, bc, :], AF.Copy,
                                 scale=rcs[:, bc:bc + 1])

        # ---- expert-0 layer 3 (the softmax chain drains meanwhile) ----
        for bh in range(BH):
            for hc in range(H3CH):
                ps = psP.tile([128, NB], F32, tag="ps")
                for kc in range(H1CH):
                    nc.tensor.matmul(ps, w3t0[:, kc, ts(hc, 128)], h2t[:, kc, ts(bh, NB)],
                                     start=(kc == 0), stop=(kc == H1CH - 1))
                nc.scalar.activation(h3t[:, hc, ts(bh, NB)], ps, AF.Relu,
                                     bias=ebt[:, 0, 8 + hc:9 + hc])

        # ---- init acc with the gate-weighted layer-4 bias: acc = gates @ B ----
        gTall = gatep.tile([E, BCH, 128], F32R)
        for bc in range(BCH):
            gps = psP.tile([E, 128], F32, name="gps", tag="ps")
            nc.tensor.transpose(gps, gates[:, bc, :], ident)
            nc.scalar.activation(gTall[:, bc, :], gps, AF.Copy)
        for bc in range(BCH):
            bps = psP.tile([128, O], F32, name="bps", tag="ps")
            nc.tensor.matmul(bps, gTall[:, bc, :], bmat, start=True, stop=True)
            nc.vector.tensor_copy(acc[:, bc, :], bps)

        def layer4(e, w4t, h3t, bcs):
            for bc in bcs:
                p4 = psP.tile([128, O], F32, name="p4", tag="ps")
                nc.tensor.matmul(p4, h3t[:, 0, ts(bc, 128)], w4t[:, 0, :],
                                 start=True, stop=False)
                nc.tensor.matmul(p4, h3t[:, 1, ts(bc, 128)], w4t[:, 1, :],
                                 start=False, stop=True)
                tm = tmpp.tile([128, O], F32)
                nc.scalar.activation(tm, p4, AF.Copy, scale=gates[:, bc, e:e + 1])
                nc.vector.tensor_add(acc[:, bc, :], acc[:, bc, :], tm)
                if e == E - 1:
                    q = nc.sync if bc % 2 == 0 else nc.gpsimd
                    q.dma_start(out=out_d.ap()[ts(bc, 128), :], in_=acc[:, bc, :])

        # ---- expert-0 layer 4 ----
        layer4(0, w4t0, h3t, range(BCH))

        # ---- experts 1..11 (weights software-pipelined one expert ahead) ----
        for e in range(1, E):
            w1t, w2t, w3t, w4t = wts
            if e + 1 < E:
                wts = wtiles(e + 1, nc.sync, nc.gpsimd)

            h1t = actp.tile([128, H1CH, BL], F32R, name="h1t")
            for bh in range(BH):            # layer 1: [1024] -> [512]
                for hc in range(H1CH):
                    ps = psP.tile([128, NB], F32, tag="ps")
                    for dc in range(DCH):
                        nc.tensor.matmul(ps, w1t[:, dc, ts(hc, 128)],
                                         xtg[:, dc, ts(bh, NB)],
                                         start=(dc == 0), stop=(dc == DCH - 1))
                    nc.vector.tensor_scalar(h1t[:, hc, ts(bh, NB)], ps,
                                            ebt[:, e, hc:hc + 1], 0.0,
                                            ALU.add, ALU.max)
            h2t = actp.tile([128, H1CH, BL], F32R, name="h2t")
            h3t = actp.tile([128, H3CH, BL], F32R, name="h3t")
            for bh in range(BH):            # layer 2: [512] -> [512]
                for hc in range(H1CH):
                    ps = psP.tile([128, NB], F32, tag="ps")
                    for kc in range(H1CH):
                        nc.tensor.matmul(ps, w2t[:, kc, ts(hc, 128)], h1t[:, kc, ts(bh, NB)],
                                         start=(kc == 0), stop=(kc == H1CH - 1))
                    nc.scalar.activation(h2t[:, hc, ts(bh, NB)], ps, AF.Relu,
                                         bias=ebt[:, e, 4 + hc:5 + hc])
            for bh in range(BH):            # layer 3: [512] -> [256]
                for hc in range(H3CH):
                    ps = psP.tile([128, NB], F32, tag="ps")
                    for kc in range(H1CH):
                        nc.tensor.matmul(ps, w3t[:, kc, ts(hc, 128)], h2t[:, kc, ts(bh, NB)],
                                         start=(kc == 0), stop=(kc == H1CH - 1))
                    nc.scalar.activation(h3t[:, hc, ts(bh, NB)], ps, AF.Relu,
                                         bias=ebt[:, e, 8 + hc:9 + hc])
                if e == E - 1 and bh == 0:  # drain first-half L4 early
                    layer4(e, w4t, h3t, range(BCH // 2))
            if e == E - 1:
                layer4(e, w4t, h3t, range(BCH // 2, BCH))
            else:
                layer4(e, w4t, h3t, range(BCH))

    nc.compile()
    return nc


def _tile128(w):
    """[K, N] -> [128, K//128, N] with per-partition-contiguous bytes."""
    k, n = w.shape
    return np.ascontiguousarray(w.reshape(k // 128, 128, n).transpose(1, 0, 2))


def _fold(inputs):
    """Fold BatchNorms into next-layer weights/biases (float64 for exactness)."""
    f = {k: np.asarray(v, dtype=np.float64) for k, v in inputs.items()}

    def sb(g, b, m, v):
        s = g / np.sqrt(v + EPS)
        return s, b - m * s

    out = {}
    # gate
    sg1, tg1 = sb(f["gbn1_g"], f["gbn1_b"], f["gbn1_m"], f["gbn1_v"])
    sg2, tg2 = sb(f["gbn2_g"], f["gbn2_b"], f["gbn2_m"], f["gbn2_v"])
    gw1t = _tile128(f["gw1"])                     # [128, DCH, 256]
    gb1c = f["gb1"]
    gw2t = _tile128(sg1[:, None] * f["gw2"])      # [128, 2, 128]
    gb2c = f["gb2"] + tg1 @ f["gw2"]
    gw3t = sg2[:, None] * f["gw3"]                # [128, E]
    gb3r = f["gb3"] + tg2 @ f["gw3"]
    # experts
    s1, t1 = sb(f["ebn1_g"], f["ebn1_b"], f["ebn1_m"], f["ebn1_v"])   # [E,H]
    s2, t2 = sb(f["ebn2_g"], f["ebn2_b"], f["ebn2_m"], f["ebn2_v"])   # [E,H]
    s3, t3 = sb(f["ebn3_g"], f["ebn3_b"], f["ebn3_m"], f["ebn3_v"])   # [E,H/2]
    out["w1"] = np.stack([_tile128(f["ew1"][e]) for e in range(E)])
    b1 = f["eb1"]                                                     # [E,H]
    out["w2"] = np.stack([_tile128(s1[e][:, None] * f["ew2"][e]) for e in range(E)])
    b2 = f["eb2"] + np.einsum("eh,eho->eo", t1, f["ew2"])
    out["w3"] = np.stack([_tile128(s2[e][:, None] * f["ew3"][e]) for e in range(E)])
    b3 = f["eb3"] + np.einsum("eh,eho->eo", t2, f["ew3"])
    out["w4"] = np.stack([_tile128(s3[e][:, None] * f["ew4"][e]) for e in range(E)])
    b4 = f["eb4"] + np.einsum("eh,eho->eo", t3, f["ew4"])
    # packed activation-bias columns: [E, 128, 10]
    eb = np.zeros((E, 128, 10))
    eb[:, :, 0:4] = b1.reshape(E, 4, 128).transpose(0, 2, 1)
    eb[:, :, 4:8] = b2.reshape(E, 4, 128).transpose(0, 2, 1)
    eb[:, :, 8:10] = b3.reshape(E, 2, 128).transpose(0, 2, 1)
    out["eb"] = eb.transpose(1, 0, 2)             # [128, E, 10]
    gwb = np.zeros((128, 2304))
    gwb[:, 0:2048] = gw1t.reshape(128, 2048)
    gwb[:, 2048:2304] = gw2t.reshape(128, 256)
    out["gwb"] = gwb
    out["w1e0"] = out["w1"][0].transpose(1, 0, 2)  # dc-major expert-0 w1
    pkr = np.zeros((128, 408))
    pkr[:1, 0:128] = 1.0                          # ones row
    pkr[:, 128:140] = gw3t
    pkr[:1, 140:152] = gb3r
    pkr[:E, 152:408] = b4
    out["pkr"] = pkr
    pkf = np.zeros((128, 131))
    pkf[:, 0:2] = gb1c.reshape(2, 128).T
    pkf[:, 2:3] = gb2c.reshape(1, 128).T
    pkf[:, 3:131] = np.eye(128)
    out["pkf"] = pkf
    return {k: np.ascontiguousarray(v, dtype=np.float32) for k, v in out.items()}


_CACHE = {}


def build_in_maps(inputs):
    w = _fold(inputs)
    xt_full = np.asarray(inputs["x"], dtype=np.float32).T               # [D, B]
    in_maps = []
    for c in range(NCORES):
        m = dict(w)
        m["xt"] = np.ascontiguousarray(
            xt_full[:, c * BL:(c + 1) * BL].reshape(DCH, 128, BL))
        in_maps.append(m)

    return in_maps


def kernel(**inputs) -> np.ndarray:
    if "nc" not in _CACHE:
        _CACHE["nc"] = _build_bass()
    nc = _CACHE["nc"]

    in_maps = build_in_maps(inputs)
    res = run_bass_kernel_spmd(nc, in_maps, core_ids=list(range(NCORES)))
    return np.concatenate([r["out"] for r in res.results], axis=0)


# revision 20
# speedup vs baseline: 1.1367x; 1.0280x over previous
"""Trainium2 Bass kernel for nn_MixtureOfExperts (B=8192, D=1024, E=12, H=512, O=256).

Strategy:
- Data-parallel over 8 NeuronCores: each core processes 1024 rows of x; all
  weights replicated. Host gathers/concats core outputs.
- Host-side prep: eval-mode BatchNorm (which follows each ReLU) is folded into
  the NEXT layer's weights and bias:  bn(relu(z)) = s*relu(z) + t  with
  s = g/sqrt(v+eps) > 0, t = b - m*s, so
      bn(relu(z)) @ W + c  ==  relu(z) @ (diag(s) W) + (c + t @ W).
  x is pre-transposed and all weights pre-tiled on host into the exact SBUF
  layout ([128 part, chunk, free] with per-partition-contiguous DRAM bytes) so
  every big DMA is a fully contiguous copy.
- All matmuls fp32r (full PE rate with moving free >= 256; measured faster
  than bf16 on this part: bf16 N=512 matmuls issue at ~250ns vs ~228ns fp32r).
- Startup is DMA-bound, so the PE is fed with real work in DMA-arrival order:
  expert-0 L1 runs FIRST in dc-streaming order with 8 open PSUM accumulation
  groups, consuming (x[dc], w1[dc]) chunk pairs as they land (each pair split
  across the two DMA queues).  Two fp32 filler matmuls cover the
  pre-first-chunk gap so the HAM clock-gate stays released.  The gate g1 then
  runs dc-streamed off per-chunk gwb slices, and the gate tail (g2/g3/softmax/
  bias-init) is interleaved between expert-0's L2/L3/L4 so the PE never waits
  on the softmax ACT/DVE chain.
- Layers 1-3 feature-major; layer 4 batch-major (stationary = h3T slice);
  gate prob applied as per-partition scalar on ScalarE, experts accumulated
  on VectorE into acc, pre-initialized with sum_e gate_e * bias4_e via a
  PE-transposed-gates matmul.  For the last expert, L4 of the first batch
  half is interleaved between the two L3 halves and the final output DMAs
  alternate between both queues, shortening the drain tail.
"""

import numpy as np
import ml_dtypes
from contextlib import ExitStack

import concourse.bass as bass
import concourse.mybir as mybir
import concourse.tile as tile
from concourse import bacc
from concourse.bass import ts
from concourse.bass_utils import run_bass_kernel_spmd

B, D, E, H, O = 8192, 1024, 12, 512, 256
NCORES = 8
BL = B // NCORES          # 1024 batch rows per core
EPS = 1e-5
F32 = mybir.dt.float32
F32R = mybir.dt.float32r
BF16 = mybir.dt.bfloat16
AF = mybir.ActivationFunctionType
ALU = mybir.AluOpType
AX = mybir.AxisListType
NPBF16 = ml_dtypes.bfloat16

DCH = D // 128            # 8  d-chunks
H1CH = H // 128           # 4  h1-chunks
H3CH = (H // 2) // 128    # 2  h3-chunks
BCH = BL // 128           # 8  b-chunks of 128
BH = BL // 512            # 2  b-halves of 512
NB = 512                  # moving free dim for layers 1-3
NFILL = 2                 # fp32 filler matmuls before the first x chunk lands


def _build_bass():
    nc = bacc.Bacc("TRN2", target_bir_lowering=False, debug=False,
                   enable_asserts=False, num_devices=NCORES)

    xt_d = nc.dram_tensor("xt", [DCH, 128, BL], F32R, kind="ExternalInput")
    # bf16 duplicates of x / expert-0 w1 / gate weights: the startup window is
    # DMA-bound, so expert-0's L1 and the gate run off half-width copies
    # (~3.1 MB critical bytes instead of ~7.3 MB) while the fp32 x for experts
    # 1-11 streams in the background.  The dc-major w1e0 layout makes the
    # startup per-chunk DMAs fully contiguous.
    xtbf_d = nc.dram_tensor("xtbf", [DCH, 128, BL], BF16, kind="ExternalInput")
    w1e0_d = nc.dram_tensor("w1e0", [DCH, 128, H], BF16, kind="ExternalInput")
    w1_d = nc.dram_tensor("w1", [E, 128, DCH, H], F32R, kind="ExternalInput")
    w2_d = nc.dram_tensor("w2", [E, 128, H1CH, H], F32R, kind="ExternalInput")
    w3_d = nc.dram_tensor("w3", [E, 128, H1CH, H // 2], F32R, kind="ExternalInput")
    w4_d = nc.dram_tensor("w4", [E, 128, H3CH, O], F32R, kind="ExternalInput")
    eb_d = nc.dram_tensor("eb", [128, E, 10], F32, kind="ExternalInput")
    # packed small constants:
    #   gwb (bf16) cols: [0:2048 gw1 (dc-major) | 2048:2304 gw2]
    #   pkr (f32r) cols: [0:128 ones | 128:140 gw3 | 140:152 gb3 | 152:408 bmat]
    #   pkf (f32)  cols: [0:2 gb1 | 2:3 gb2 | 3:131 ident]
    gwb_d = nc.dram_tensor("gwb", [128, 2304], BF16, kind="ExternalInput")
    pkr_d = nc.dram_tensor("pkr", [128, 408], F32R, kind="ExternalInput")
    pkf_d = nc.dram_tensor("pkf", [128, 131], F32, kind="ExternalInput")
    out_d = nc.dram_tensor("out", [BL, O], F32, kind="ExternalOutput")

    with tile.TileContext(nc) as tc, ExitStack() as ctx:
        const = ctx.enter_context(tc.tile_pool(name="const", bufs=1))
        gatep = ctx.enter_context(tc.tile_pool(name="gatep", bufs=1))
        gtmp = ctx.enter_context(tc.tile_pool(name="gtmp", bufs=2))
        wpool = ctx.enter_context(tc.tile_pool(name="wpool", bufs=2))
        actp = ctx.enter_context(tc.tile_pool(name="actp", bufs=1))
        accp = ctx.enter_context(tc.tile_pool(name="accp", bufs=1))
        tmpp = ctx.enter_context(tc.tile_pool(name="tmpp", bufs=4))
        # single 8-bank PSUM ring (every slot sized [128, 512] fp32 = 1 bank)
        psP = ctx.enter_context(tc.tile_pool(name="psP", bufs=8, space="PSUM"))

        # ---- startup DMAs in strict consumption order on two queues ----
        # Every transfer here reads fully-contiguous DRAM (big packets).
        # (xbf[dc], w1bf[dc]) pairs stream on opposite queues so the
        # dc-milestones land in lockstep; everything else follows.
        scr = const.tile([128, 512], F32)
        nc.vector.memset(scr, 0.0)
        pkf = const.tile([128, 131], F32)
        nc.sync.dma_start(out=pkf, in_=pkf_d.ap())
        ebt = const.tile([128, E, 10], F32)
        nc.gpsimd.dma_start(out=ebt, in_=eb_d.ap())
        xbf = const.tile([128, DCH, BL], BF16)
        w1bf = const.tile([128, DCH, H], BF16)
        for dc in range(DCH):
            qx = nc.sync if dc % 2 == 0 else nc.gpsimd
            qw = nc.gpsimd if dc % 2 == 0 else nc.sync
            qx.dma_start(out=xbf[:, dc], in_=xtbf_d.ap()[dc])
            qw.dma_start(out=w1bf[:, dc], in_=w1e0_d.ap()[dc])
        gwb = const.tile([128, 2304], BF16)
        nc.sync.dma_start(out=gwb[:, 0:1152], in_=gwb_d.ap()[:, 0:1152])
        nc.gpsimd.dma_start(out=gwb[:, 1152:2304], in_=gwb_d.ap()[:, 1152:2304])
        w2t0 = wpool.tile([128, H1CH, H], F32R, name="w2t")
        nc.sync.dma_start(out=w2t0, in_=w2_d.ap()[0])
        w3t0 = wpool.tile([128, H1CH, H // 2], F32R, name="w3t")
        nc.gpsimd.dma_start(out=w3t0, in_=w3_d.ap()[0])
        w4t0 = wpool.tile([128, H3CH, O], F32R, name="w4t")
        nc.gpsimd.dma_start(out=w4t0, in_=w4_d.ap()[0])
        pkr = const.tile([128, 408], F32R)
        nc.sync.dma_start(out=pkr, in_=pkr_d.ap())
        # fp32 x for experts 1-11 streams behind the startup-critical bytes
        xtg = const.tile([128, DCH, BL], F32R)
        for dc in range(DCH):
            eng = nc.sync if dc % 2 == 0 else nc.gpsimd
            eng.dma_start(out=xtg[:, dc], in_=xt_d.ap()[dc])

        def wtiles(e, q1, q2):
            w1t = wpool.tile([128, DCH, H], F32R, name="w1t")
            q1.dma_start(out=w1t[:, :DCH // 2], in_=w1_d.ap()[e, :, :DCH // 2])
            q2.dma_start(out=w1t[:, DCH // 2:], in_=w1_d.ap()[e, :, DCH // 2:])
            w2t = wpool.tile([128, H1CH, H], F32R, name="w2t")
            q1.dma_start(out=w2t, in_=w2_d.ap()[e])
            w3t = wpool.tile([128, H1CH, H // 2], F32R, name="w3t")
            q2.dma_start(out=w3t, in_=w3_d.ap()[e])
            w4t = wpool.tile([128, H3CH, O], F32R, name="w4t")
            q1.dma_start(out=w4t, in_=w4_d.ap()[e])
            return w1t, w2t, w3t, w4t

        gw1v = gwb[:, 0:2048].rearrange("p (c m) -> p c m", c=DCH)
        gw2v = gwb[:, 2048:2304].rearrange("p (c m) -> p c m", c=2)
        ones = pkr[:1, 0:128]
        gw3 = pkr[:, 128:140]
        gb3 = pkr[:1, 140:152]
        bmat = pkr[:E, 152:408]
        gb1 = pkf[:, 0:2]
        gb2 = pkf[:, 2:3]
        ident = pkf[:, 3:131]
        acc = accp.tile([128, BCH, O], F32)

        # ---- PE filler: keep the HAM clock-gate released until x[0] lands ----
        for _ in range(NFILL):
            wps = psP.tile([128, 512], F32, name="fill", tag="ps")
            nc.tensor.matmul(wps, scr[:, :128], scr, start=True, stop=True)

        # ---- expert-0 layer 1, dc-streaming: 8 open PSUM groups consume
        #      (xbf[dc], w1bf[dc]) chunk pairs as the DMAs land ----
        h1t = actp.tile([128, H1CH, BL], F32R, name="h1t")
        ps_l1 = [psP.tile([128, NB], F32, name=f"l1g{g}", tag="ps") for g in range(8)]
        for dc in range(DCH):
            for bh in range(BH):
                for hc in range(H1CH):
                    nc.tensor.matmul(ps_l1[bh * H1CH + hc],
                                     w1bf[:, dc, ts(hc, 128)],
                                     xbf[:, dc, ts(bh, NB)],
                                     start=(dc == 0), stop=(dc == DCH - 1))
        for bh in range(BH):
            for hc in range(H1CH):
                nc.vector.tensor_scalar(h1t[:, hc, ts(bh, NB)],
                                        ps_l1[bh * H1CH + hc],
                                        ebt[:, 0, hc:hc + 1], 0.0,
                                        ALU.add, ALU.max)

        # ---- gate g1 (bf16; everything resident by now) ----
        g1t = gatep.tile([128, 2, BL], BF16)
        g2t = gatep.tile([128, BL], F32R)
        gates = gatep.tile([128, BCH, E], F32)
        ps_g1 = [psP.tile([128, NB], F32, name=f"g1g{g}", tag="ps") for g in range(4)]
        for dc in range(DCH):
            for bh in range(BH):
                for hc in range(2):
                    nc.tensor.matmul(ps_g1[bh * 2 + hc],
                                     gw1v[:, dc, ts(hc, 128)],
                                     xbf[:, dc, ts(bh, NB)],
                                     start=(dc == 0), stop=(dc == DCH - 1))
        for bh in range(BH):
            for hc in range(2):
                nc.scalar.activation(g1t[:, hc, ts(bh, NB)], ps_g1[bh * 2 + hc],
                                     AF.Relu, bias=gb1[:, hc:hc + 1])

        # prefetch expert-1 weights behind the startup stream
        wts = wtiles(1, nc.sync, nc.gpsimd)

        # ---- expert-0 layer 2 (dense; overlaps gate-tail ACT/DVE work) ----
        h2t = actp.tile([128, H1CH, BL], F32R, name="h2t")
        h3t = actp.tile([128, H3CH, BL], F32R, name="h3t")
        for bh in range(BH):
            for hc in range(H1CH):
                ps = psP.tile([128, NB], F32, tag="ps")
                for kc in range(H1CH):
                    nc.tensor.matmul(ps, w2t0[:, kc, ts(hc, 128)], h1t[:, kc, ts(bh, NB)],
                                     start=(kc == 0), stop=(kc == H1CH - 1))
                nc.scalar.activation(h2t[:, hc, ts(bh, NB)], ps, AF.Relu,
                                     bias=ebt[:, 0, 4 + hc:5 + hc])

        # ---- gate g2/g3 + softmax ----
        for bh in range(BH):
            ps = psP.tile([128, NB], F32, tag="ps")
            for kc in range(2):
                nc.tensor.matmul(ps, gw2v[:, kc, :], g1t[:, kc, ts(bh, NB)],
                                 start=(kc == 0), stop=(kc == 1))
            nc.scalar.activation(g2t[:, ts(bh, NB)], ps, AF.Relu, bias=gb2[:, 0:1])
        psgall = psP.tile([128, BCH, E], F32, name="psgall", tag="ps")
        for bc in range(BCH):
            nc.tensor.matmul(psgall[:, bc, :], g2t[:, ts(bc, 128)], gw3,
                             start=True, stop=False)
            nc.tensor.matmul(psgall[:, bc, :], ones[:1, :], gb3[:1, :],
                             start=False, stop=True)
        exall = gatep.tile([128, BCH, E], F32)
        nc.scalar.activation(exall, psgall, AF.Exp)
        sms = gtmp.tile([128, BCH], F32)
        nc.vector.tensor_reduce(sms, exall, AX.X, ALU.add)
        rcs = gtmp.tile([128, BCH], F32)
        nc.vector.reciprocal(rcs, sms)
        for bc in range(BCH):
            nc.scalar.activation(gates[:, bc, :], exall[:, bc, :], AF.Copy,
                                 scale=rcs[:, bc:bc + 1])

        # ---- expert-0 layer 3 (the softmax chain drains meanwhile) ----
        for bh in range(BH):
            for hc in range(H3CH):
                ps = psP.tile([128, NB], F32, tag="ps")
                for kc in range(H1CH):
                    nc.tensor.matmul(ps, w3t0[:, kc, ts(hc, 128)], h2t[:, kc, ts(bh, NB)],
                                     start=(kc == 0), stop=(kc == H1CH - 1))
                nc.scalar.activation(h3t[:, hc, ts(bh, NB)], ps, AF.Relu,
                                     bias=ebt[:, 0, 8 + hc:9 + hc])

        # ---- init acc with the gate-weighted layer-4 bias: acc = gates @ B ----
        gTall = gatep.tile([E, BCH, 128], F32R)
        for bc in range(BCH):
            gps = psP.tile([E, 128], F32, name="gps", tag="ps")
            nc.tensor.transpose(gps, gates[:, bc, :], ident)
            nc.scalar.activation(gTall[:, bc, :], gps, AF.Copy)
        for bc in range(BCH):
            bps = psP.tile([128, O], F32, name="bps", tag="ps")
            nc.tensor.matmul(bps, gTall[:, bc, :], bmat, start=True, stop=True)
            nc.vector.tensor_copy(acc[:, bc, :], bps)

        def layer4(e, w4t, h3t, bcs):
            for bc in bcs:
                p4 = psP.tile([128, O], F32, name="p4", tag="ps")
                nc.tensor.matmul(p4, h3t[:, 0, ts(bc, 128)], w4t[:, 0, :],
                                 start=True, stop=False)
                nc.tensor.matmul(p4, h3t[:, 1, ts(bc, 128)], w4t[:, 1, :],
                                 start=False, stop=True)
                tm = tmpp.tile([128, O], F32)
                if e == E - 1 and bc % 2 == 1:
                    # final drain: alternate engines so the scale+add chains
                    # of adjacent bc's run in parallel
                    nc.vector.tensor_scalar(tm, p4, gates[:, bc, e:e + 1], 0.0,
                                            ALU.mult, ALU.add)
                    nc.gpsimd.tensor_tensor(acc[:, bc, :], acc[:, bc, :], tm,
                                            ALU.add)
                else:
                    nc.scalar.activation(tm, p4, AF.Copy, scale=gates[:, bc, e:e + 1])
                    nc.vector.tensor_add(acc[:, bc, :], acc[:, bc, :], tm)
                if e == E - 1:
                    q = nc.sync if bc % 2 == 0 else nc.gpsimd
                    q.dma_start(out=out_d.ap()[ts(bc, 128), :], in_=acc[:, bc, :])

        # ---- expert-0 layer 4 ----
        layer4(0, w4t0, h3t, range(BCH))

        # ---- experts 1..11 (weights software-pipelined one expert ahead) ----
        for e in range(1, E):
            w1t, w2t, w3t, w4t = wts
            if e + 1 < E:
                wts = wtiles(e + 1, nc.sync, nc.gpsimd)

            h1t = actp.tile([128, H1CH, BL], F32R, name="h1t")
            for bh in range(BH):            # layer 1: [1024] -> [512]
                for hc in range(H1CH):
                    ps = psP.tile([128, NB], F32, tag="ps")
                    for dc in range(DCH):
                        nc.tensor.matmul(ps, w1t[:, dc, ts(hc, 128)],
                                         xtg[:, dc, ts(bh, NB)],
                                         start=(dc == 0), stop=(dc == DCH - 1))
                    nc.vector.tensor_scalar(h1t[:, hc, ts(bh, NB)], ps,
                                            ebt[:, e, hc:hc + 1], 0.0,
                                            ALU.add, ALU.max)
            h2t = actp.tile([128, H1CH, BL], F32R, name="h2t")
            h3t = actp.tile([128, H3CH, BL], F32R, name="h3t")
            for bh in range(BH):            # layer 2: [512] -> [512]
                for hc in range(H1CH):
                    ps = psP.tile([128, NB], F32, tag="ps")
                    for kc in range(H1CH):
                        nc.tensor.matmul(ps, w2t[:, kc, ts(hc, 128)], h1t[:, kc, ts(bh, NB)],
                                         start=(kc == 0), stop=(kc == H1CH - 1))
                    nc.scalar.activation(h2t[:, hc, ts(bh, NB)], ps, AF.Relu,
                                         bias=ebt[:, e, 4 + hc:5 + hc])
            for bh in range(BH):            # layer 3: [512] -> [256]
                for hc in range(H3CH):
                    ps = psP.tile([128, NB], F32, tag="ps")
                    for kc in range(H1CH):
                        nc.tensor.matmul(ps, w3t[:, kc, ts(hc, 128)], h2t[:, kc, ts(bh, NB)],
                                         start=(kc == 0), stop=(kc == H1CH - 1))
                    nc.scalar.activation(h3t[:, hc, ts(bh, NB)], ps, AF.Relu,
                                         bias=ebt[:, e, 8 + hc:9 + hc])
                if e == E - 1 and bh == 0:  # drain first-half L4 early
                    layer4(e, w4t, h3t, range(BCH // 2))
            if e == E - 1:
                layer4(e, w4t, h3t, range(BCH // 2, BCH))
            else:
                layer4(e, w4t, h3t, range(BCH))

    nc.compile()
    return nc


def _tile128(w):
    """[K, N] -> [128, K//128, N] with per-partition-contiguous bytes."""
    k, n = w.shape
    return np.ascontiguousarray(w.reshape(k // 128, 128, n).transpose(1, 0, 2))


def _fold(inputs):
    """Fold BatchNorms into next-layer weights/biases (float64 for exactness)."""
    f = {k: np.asarray(v, dtype=np.float64) for k, v in inputs.items()}

    def sb(g, b, m, v):
        s = g / np.sqrt(v + EPS)
        return s, b - m * s

    out = {}
    # gate
    sg1, tg1 = sb(f["gbn1_g"], f["gbn1_b"], f["gbn1_m"], f["gbn1_v"])
    sg2, tg2 = sb(f["gbn2_g"], f["gbn2_b"], f["gbn2_m"], f["gbn2_v"])
    gw1t = _tile128(f["gw1"])                     # [128, DCH, 256]
    gb1c = f["gb1"]
    gw2t = _tile128(sg1[:, None] * f["gw2"])      # [128, 2, 128]
    gb2c = f["gb2"] + tg1 @ f["gw2"]
    gw3t = sg2[:, None] * f["gw3"]                # [128, E]
    gb3r = f["gb3"] + tg2 @ f["gw3"]
    # experts
    s1, t1 = sb(f["ebn1_g"], f["ebn1_b"], f["ebn1_m"], f["ebn1_v"])   # [E,H]
    s2, t2 = sb(f["ebn2_g"], f["ebn2_b"], f["ebn2_m"], f["ebn2_v"])   # [E,H]
    s3, t3 = sb(f["ebn3_g"], f["ebn3_b"], f["ebn3_m"], f["ebn3_v"])   # [E,H/2]
    out["w1"] = np.stack([_tile128(f["ew1"][e]) for e in range(E)])
    b1 = f["eb1"]                                                     # [E,H]
    out["w2"] = np.stack([_tile128(s1[e][:, None] * f["ew2"][e]) for e in range(E)])
    b2 = f["eb2"] + np.einsum("eh,eho->eo", t1, f["ew2"])
    out["w3"] = np.stack([_tile128(s2[e][:, None] * f["ew3"][e]) for e in range(E)])
    b3 = f["eb3"] + np.einsum("eh,eho->eo", t2, f["ew3"])
    out["w4"] = np.stack([_tile128(s3[e][:, None] * f["ew4"][e]) for e in range(E)])
    b4 = f["eb4"] + np.einsum("eh,eho->eo", t3, f["ew4"])
    # packed activation-bias columns: [E, 128, 10]
    eb = np.zeros((E, 128, 10))
    eb[:, :, 0:4] = b1.reshape(E, 4, 128).transpose(0, 2, 1)
    eb[:, :, 4:8] = b2.reshape(E, 4, 128).transpose(0, 2, 1)
    eb[:, :, 8:10] = b3.reshape(E, 2, 128).transpose(0, 2, 1)
    out["eb"] = eb.transpose(1, 0, 2)             # [128, E, 10]
    gwb = np.zeros((128, 2304))
    gwb[:, 0:2048] = gw1t.reshape(128, 2048)
    gwb[:, 2048:2304] = gw2t.reshape(128, 256)
    bfs = {"gwb": gwb,
           "w1e0": out["w1"][0].transpose(1, 0, 2)}  # dc-major expert-0 w1
    pkr = np.zeros((128, 408))
    pkr[:1, 0:128] = 1.0                          # ones row
    pkr[:, 128:140] = gw3t
    pkr[:1, 140:152] = gb3r
    pkr[:E, 152:408] = b4
    out["pkr"] = pkr
    pkf = np.zeros((128, 131))
    pkf[:, 0:2] = gb1c.reshape(2, 128).T
    pkf[:, 2:3] = gb2c.reshape(1, 128).T
    pkf[:, 3:131] = np.eye(128)
    out["pkf"] = pkf
    res = {k: np.ascontiguousarray(v, dtype=np.float32) for k, v in out.items()}
    for k, v in bfs.items():
        res[k] = np.ascontiguousarray(v, dtype=NPBF16)
    return res


_CACHE = {}


def build_in_maps(inputs):
    w = _fold(inputs)
    xt_full = np.asarray(inputs["x"], dtype=np.float32).T               # [D, B]
    in_maps = []
    for c in range(NCORES):
        m = dict(w)
        m["xt"] = np.ascontiguousarray(
            xt_full[:, c * BL:(c + 1) * BL].reshape(DCH, 128, BL))
        m["xtbf"] = np.ascontiguousarray(m["xt"].astype(NPBF16))
        in_maps.append(m)

    return in_maps


def kernel(**inputs) -> np.ndarray:
    if "nc" not in _CACHE:
        _CACHE["nc"] = _build_bass()
    nc = _CACHE["nc"]

    in_maps = build_in_maps(inputs)
    res = run_bass_kernel_spmd(nc, in_maps, core_ids=list(range(NCORES)))
    return np.concatenate([r["out"] for r in res.results], axis=0)
